# revision 10
# baseline (speedup 1.0000x reference)
"""Trainium2 Bass kernel: single-head causal attention with RoPE,
sharded across 8 NeuronCores (2 cores per batch element).

kernel(**inputs) takes the FULL inputs (x [4,4096,1024], wq/wk/wv
[1024,1024], all fp32) and returns the FULL output [4,4096,1024] fp32.

Sharding: core c handles batch b=c//2; parity h=c%2 selects both its
query superblocks (g_map pairing, identical extents 512*(i+1) on both
cores) and its key blocks (the 256-key blocks with global index 2j+h).
Each core projects K/V only for its own 8 key blocks (half the work),
processed as four 512-key double-blocks; each double-block's K^T/V is
exchanged pairwise with its own AllGather so the wire overlaps both the
remaining projections and the early attention superblocks.

Device layouts (partition dim first, rank-uniform so the program is
identical on every core):
  KT_sb [128, 8, 2, 8, 256]: K^T; dims = (feat%128, feat-chunk,
        rank-parity r, rank-local block B, key-in-block).  Global
        256-key block Gb = 2B + r.
  V_sb  [128, 2, 16, 1025]: V; dims = (key%128, rank-parity r,
        rank-local 128-key chunk 2B+half, feature).  Col 1024 = 1.0
        (softmax denominator rides the PV matmul).
  QT    [128, 8, 256] per-superblock Q^T (RoPE'd, perm)
Scores are computed transposed (S^T[k,q]) so P^T = exp(S^T) feeds the
PV matmul as the stationary operand.  Softmax runs without
max-subtraction (scores are O(6); exp is safe in fp32).

Engine budget: PE does all matmuls; DVE does RoPE directly from PSUM at
[128,512] granularity (and the V psum evictions) so the scalar engine
only runs input-stream DMAs and the phase-C exp activations; gpsimd
owns the collective triggers and the gather landing DMAs (interleaved
one chunk behind the triggers so a blocked landing never delays the
next trigger by more than one AG).
"""

import sys
for _p in ("/root/.axon_site", "/root/.axon_site/_ro/trn_rl_repo",
           "/root/.axon_site/_ro/pypackages"):
    if _p not in sys.path:
        sys.path.append(_p)

import numpy as np

import concourse.bass as bass
import concourse.bacc as bacc
import concourse.mybir as mybir
from concourse import tile

P = 128
D = 1024
DC = 8          # feature chunks of 128
W = 256         # key-block width
SUP_Q = 256     # q rows per superblock
NSUP = 8
NBLK = 8        # own 256-key blocks per core
NDB = 4         # own 512-key double-blocks per core (one AllGather each)
ROPE_BASE = 10000.0
SCALE = 1.0 / 32.0   # 1/sqrt(D)
DV1 = 1025      # V free width incl. ones column
PV_SPLIT = [(0, 342), (342, 684), (684, 1025)]
# KT column order inside a staged chunk: [se0, so0, se1, so1, ...] so each
# rope output pair is one contiguous [P,1024] store; landing looks up the
# feature-chunk position with this map.
KT_COL = [0, 2, 4, 6, 1, 3, 5, 7]

BF = mybir.dt.bfloat16
F32 = mybir.dt.float32
AF = mybir.ActivationFunctionType


def build_kernel(nc, SEQ):
    QROWS = SEQ // 2

    xk = nc.dram_tensor("xk", [NDB, P, DC, 2 * W], BF, kind="ExternalInput")
    xq = nc.dram_tensor("xq", [NSUP, P, DC, SUP_Q], BF, kind="ExternalInput")
    wkT = nc.dram_tensor("wkT", [D, D], BF, kind="ExternalInput")
    wqT = nc.dram_tensor("wqT", [D, D], BF, kind="ExternalInput")
    wvT = nc.dram_tensor("wvT", [D, D], BF, kind="ExternalInput")
    csk = nc.dram_tensor("csk", [NDB, P, 8, 2 * W], BF, kind="ExternalInput")
    csq = nc.dram_tensor("csq", [NSUP, P, 8, SUP_Q], BF, kind="ExternalInput")
    maskI = nc.dram_tensor("mask", [NSUP, P, 4, SUP_Q], BF, kind="ExternalInput")
    out = nc.dram_tensor("out", [QROWS, D], F32, kind="ExternalOutput")

    wk_r = wkT.rearrange("(c p) e -> p c e", p=P)
    wq_r = wqT.rearrange("(c p) e -> p c e", p=P)
    wv_r = wvT.rearrange("(c p) e -> p c e", p=P)

    # per-double-block AG bounce buffers:
    # cols [0:4096] = K^T (8 x 512 in KT_COL order), [4096:8192] = V
    couts = [nc.dram_tensor(f"cout{j}", [P, 8192], BF) for j in range(NDB)]
    cgs = [nc.dram_tensor(f"cg{j}", [2, P, 8192], BF) for j in range(NDB)]
    ngroups = max(1, nc.num_devices // 2)
    groups = [[2 * g, 2 * g + 1] for g in range(ngroups)]

    with tile.TileContext(nc) as tc:
        _emit(tc, nc, xk, xq, wk_r, wq_r, wv_r, csk, csq, maskI, out,
              couts, cgs, groups)
    return nc


def _rope_evict(nc, pool, pse, pso, cos_ap, sin_ap, out_e_ap, out_o_ap, width, tag):
    """out_e = e*cos - o*sin ; out_o = o*cos + e*sin, reading PSUM directly
    on the DVE (psum fp32 -> sbuf bf16)."""
    me = pool.tile([P, width], BF, tag=f"{tag}me")
    nc.vector.tensor_mul(me[:], pse[:], cos_ap)
    mo = pool.tile([P, width], BF, tag=f"{tag}mo")
    nc.vector.tensor_mul(mo[:], pso[:], sin_ap)
    nc.vector.tensor_sub(out_e_ap, me[:], mo[:])
    me2 = pool.tile([P, width], BF, tag=f"{tag}me")
    nc.vector.tensor_mul(me2[:], pse[:], sin_ap)
    mo2 = pool.tile([P, width], BF, tag=f"{tag}mo")
    nc.vector.tensor_mul(mo2[:], pso[:], cos_ap)
    nc.vector.tensor_add(out_o_ap, me2[:], mo2[:])


def _load_w_chunked(nc, pool, w_r, tag, split=False, chunks=None):
    # split=True: chunks 0-3 on the sync ring, 4-7 on the scalar ring so the
    # full weight arrives in half the time at startup
    tiles = []
    for dc in (chunks if chunks is not None else range(DC)):
        t = pool.tile([P, D], BF, tag=f"{tag}{dc}", name=f"{tag}_{dc}")
        eng = nc.scalar if (split and dc >= DC // 2) else nc.sync
        eng.dma_start(t[:], w_r[:, dc, :])
        tiles.append(t)
    return tiles


def _emit(tc, nc, xk, xq, wk_r, wq_r, wv_r, csk, csq, maskI, out,
          couts, cgs, groups):
    with (
        tc.tile_pool(name="kt", bufs=1) as ktp,
        tc.tile_pool(name="v", bufs=1) as vp,
        tc.tile_pool(name="wq", bufs=1) as wqp,
    ):
        KT_sb = ktp.tile([P, DC, 2, NBLK, W], BF, tag="KT")
        V_sb = vp.tile([P, 2, 2 * NBLK, DV1], BF, tag="V")
        nc.vector.memset(V_sb[:, :, :, 1024:1025], 1.0)

        # ---------- Phase A: own-half K+V projections, chunked AllGather ----
        with (
            tc.tile_pool(name="wkv", bufs=1) as wkvp,
            tc.tile_pool(name="xs", bufs=2) as xsp,
            tc.tile_pool(name="cs", bufs=1) as csp,
            tc.tile_pool(name="ev", bufs=1) as evp,
            tc.tile_pool(name="kvs", bufs=6) as kvsp,
            tc.tile_pool(name="pa", bufs=6, space="PSUM") as pap,
        ):
            xt0 = xsp.tile([P, DC, 2 * W], BF, tag="xk", name="xt_0")
            nc.scalar.dma_start(xt0[:], xk[0])
            cs0 = csp.tile([P, 8, 2 * W], BF, tag="cs", name="cs_0")
            nc.scalar.dma_start(cs0[:], csk[0])
            wk_t = _load_w_chunked(nc, wkvp, wk_r, "wk", split=True)
            wv_t = _load_w_chunked(nc, wkvp, wv_r, "wv", split=True)
            wq_lo = _load_w_chunked(nc, wqp, wq_r, "wq", chunks=range(4))

            def land(j):
                cgv = cgs[j].rearrange("g p x -> p g x")
                for r in range(2):
                    for jh in range(DC):
                        c0 = KT_COL[jh] * 512
                        nc.gpsimd.dma_start(
                            KT_sb[:, jh, r, 2 * j:2 * j + 2, :],
                            cgv[:, r, c0:c0 + 512])
                    nc.gpsimd.dma_start(
                        V_sb[:, r, 4 * j:4 * j + 4, 0:1024],
                        cgv[:, r, 4096:8192])

            for db in range(NDB):
                cout = couts[db]
                if db == 0:
                    xt, cst = xt0, cs0
                else:
                    xt = xsp.tile([P, DC, 2 * W], BF, tag="xk", name=f"xt_{db}")
                    nc.scalar.dma_start(xt[:], xk[db])
                    cst = csp.tile([P, 8, 2 * W], BF, tag="cs", name=f"cs_{db}")
                    nc.scalar.dma_start(cst[:], csk[db])

                # K projection: [P,512] psums, stationary shared across the
                # two 256-key halves
                for j in range(4):
                    pse = pap.tile([P, 2 * W], F32, tag="ps")
                    for dc in range(DC):
                        nc.tensor.matmul(pse[:],
                                         wk_t[dc][:, j * P:(j + 1) * P],
                                         xt[:, dc, :],
                                         start=(dc == 0), stop=(dc == DC - 1))
                    pso = pap.tile([P, 2 * W], F32, tag="ps")
                    for dc in range(DC):
                        nc.tensor.matmul(pso[:],
                                         wk_t[dc][:, (j + 4) * P:(j + 5) * P],
                                         xt[:, dc, :],
                                         start=(dc == 0), stop=(dc == DC - 1))
                    kso = kvsp.tile([P, 2, 2 * W], BF, tag="kso")
                    _rope_evict(nc, evp, pse, pso,
                                cst[:, j, :], cst[:, j + 4, :],
                                kso[:, 0, :], kso[:, 1, :], 2 * W, "k")
                    nc.sync.dma_start(
                        cout[:, j * 1024:(j + 1) * 1024], kso[:])

                # V projection
                for sc in range(4):
                    vso = kvsp.tile([P, 2, 512], BF, tag="kso")
                    for half in range(2):
                        psv = pap.tile([P, 512], F32, tag="ps")
                        for dc in range(DC):
                            nc.tensor.matmul(
                                psv[:],
                                xt[:, dc, sc * P:(sc + 1) * P],
                                wv_t[dc][:, half * 512:(half + 1) * 512],
                                start=(dc == 0), stop=(dc == DC - 1))
                        nc.vector.tensor_scalar_mul(vso[:, half, :], psv[:], 1.0)
                    base = 4096 + sc * 1024
                    nc.sync.dma_start(cout[:, base:base + 1024], vso[:])

                nc.gpsimd.collective_compute(
                    "AllGather", mybir.AluOpType.bypass,
                    replica_groups=groups, ins=[cout[:]], outs=[cgs[db][:]])
                # land this chunk right behind its own trigger: the landing
                # blocks the gpsimd queue until the AG completes, so the NEXT
                # trigger fires only then -- spreading AG 1..3 (and their HBM
                # bounce traffic) into phase C, where the HBM is otherwise
                # idle.  Only AG0 contends with the phase-A streams.
                land(db)

        # ---------- Phase C: per-superblock Q projection + attention ----------
        with (
            tc.tile_pool(name="wqh", bufs=1) as wqhp,
            tc.tile_pool(name="xqp", bufs=2) as xqp,
            tc.tile_pool(name="cq", bufs=2) as cqp,
            tc.tile_pool(name="qt", bufs=2) as qtp,
            tc.tile_pool(name="evq", bufs=3) as evqp,
            tc.tile_pool(name="pt", bufs=6) as ptp,
            tc.tile_pool(name="mk", bufs=2) as mkp,
            tc.tile_pool(name="ot", bufs=3) as otp,
            tc.tile_pool(name="rd", bufs=2) as rdp,
            tc.tile_pool(name="pq", bufs=2, space="PSUM") as pqp,
            tc.tile_pool(name="po", bufs=1, space="PSUM") as pop,
        ):
            wq_hi = _load_w_chunked(nc, wqhp, wq_r, "wqh", split=True,
                                    chunks=range(4, DC))
            wq_t = list(wq_lo) + list(wq_hi)

            def load_slot(i):
                xqt = xqp.tile([P, DC, SUP_Q], BF, tag="xq")
                nc.scalar.dma_start(xqt[:], xq[i])
                cq = cqp.tile([P, 8, SUP_Q], BF, tag="cq")
                nc.scalar.dma_start(cq[:], csq[i])
                mk = mkp.tile([P, 4, SUP_Q], BF, tag="mk")
                nc.scalar.dma_start(mk[:], maskI[i])
                return xqt, cq, mk

            nxt = load_slot(0)
            for i in range(NSUP):
                q0 = i * SUP_Q
                xqt, cq, mk = nxt
                if i + 1 < NSUP:
                    nxt = load_slot(i + 1)
                QT = qtp.tile([P, DC, SUP_Q], BF, tag="QT")
                for j in range(4):
                    pse = pqp.tile([P, SUP_Q], F32, tag="pq")
                    for dc in range(DC):
                        nc.tensor.matmul(pse[:], wq_t[dc][:, j * P:(j + 1) * P],
                                         xqt[:, dc, :],
                                         start=(dc == 0), stop=(dc == DC - 1))
                    pso = pqp.tile([P, SUP_Q], F32, tag="pq")
                    for dc in range(DC):
                        nc.tensor.matmul(pso[:], wq_t[dc][:, (j + 4) * P:(j + 5) * P],
                                         xqt[:, dc, :],
                                         start=(dc == 0), stop=(dc == DC - 1))
                    _rope_evict(nc, evqp, pse, pso,
                                cq[:, j, :], cq[:, j + 4, :],
                                QT[:, j, :], QT[:, j + 4, :],
                                SUP_Q, "q")

                nkc = 4 * (i + 1)
                o_ps = [pop.tile([P, sl[1] - sl[0]], F32, tag=f"po{n}",
                                 name=f"o_ps{i}_{n}")
                        for n, sl in enumerate(PV_SPLIT + PV_SPLIT)]

                pending = None
                for kc in range(nkc):
                    gb, half = kc // 2, kc % 2
                    r, b = gb % 2, gb // 2
                    ps_s = pqp.tile([P, SUP_Q], F32, tag="pq")
                    for dc in range(DC):
                        nc.tensor.matmul(
                            ps_s[:],
                            KT_sb[:, dc, r, b, half * P:(half + 1) * P],
                            QT[:, dc, :],
                            start=(dc == 0), stop=(dc == DC - 1))
                    pt = ptp.tile([P, SUP_Q], BF, tag="pt")
                    nc.scalar.activation(pt[:], ps_s[:], AF.Exp, scale=SCALE)
                    if kc >= nkc - 4:
                        nc.vector.tensor_mul(pt[:], pt[:], mk[:, kc - (nkc - 4), :])
                    if pending is not None:
                        _emit_pv(nc, pending, V_sb, o_ps, nkc)
                    pending = (pt, kc)
                _emit_pv(nc, pending, V_sb, o_ps, nkc)

                rd = rdp.tile([P, 2], F32, tag="rd")
                nc.vector.reciprocal(rd[:, 0:1], o_ps[2][:, 340:341])
                nc.vector.reciprocal(rd[:, 1:2], o_ps[5][:, 340:341])
                for qs in range(2):
                    ot = otp.tile([P, D], F32, tag="ot")
                    for n, (a, b) in enumerate(PV_SPLIT):
                        bb = min(b, D)
                        nc.vector.tensor_scalar_mul(
                            ot[:, a:bb], o_ps[qs * 3 + n][:, 0:bb - a],
                            rd[:, qs:qs + 1])
                    r0 = q0 + qs * P
                    nc.sync.dma_start(out[r0:r0 + P, :], ot[:])


def _emit_pv(nc, pending, V_sb, o_ps, nkc):
    pt, kc = pending
    first = (kc == 0)
    last = (kc == nkc - 1)
    gb, half = kc // 2, kc % 2
    r, b = gb % 2, gb // 2
    for qs in range(2):
        lhs = pt[:, qs * P:(qs + 1) * P]
        for n, (a, bb) in enumerate(PV_SPLIT):
            nc.tensor.matmul(o_ps[qs * 3 + n][:], lhs,
                             V_sb[:, r, 2 * b + half, a:bb],
                             start=first, stop=last)


# ---------------------------------------------------------------------------
# Host-side data preparation
# ---------------------------------------------------------------------------

def rope_tables(seq_len):
    pos = np.arange(seq_len, dtype=np.float64)
    inv = ROPE_BASE ** (-np.arange(0, D, 2, dtype=np.float64) / D)
    fr = inv[:, None] * pos[None, :]
    return np.cos(fr).astype(np.float32), np.sin(fr).astype(np.float32)


def perm_indices():
    return np.concatenate([np.arange(0, D, 2), np.arange(1, D, 2)])


def g_map(NSUP_, i, h):
    """Global superblock held in local slot i on parity-h cores.  The first
    half of slots takes parity h, the second half parity 1-h: both cores
    then need extent exactly 512*(i+1) per slot and the mask-rounding waste
    splits evenly instead of landing all on one parity."""
    return 2 * i + h if i < NSUP_ // 2 else 2 * i + (1 - h)


def q_indices(SEQ, h):
    n = SEQ // (2 * SUP_Q)
    return np.concatenate(
        [np.arange(SUP_Q * g_map(n, i, h), SUP_Q * g_map(n, i, h) + SUP_Q)
         for i in range(n)])


def k_indices(SEQ, h):
    """Own key columns: the 256-key blocks with global index 2j+h."""
    n = SEQ // (2 * W)
    return np.concatenate(
        [np.arange(W * (2 * j + h), W * (2 * j + h) + W) for j in range(n)])


def make_masks(SEQ, h):
    n = SEQ // (2 * SUP_Q)
    m = np.zeros((n, 4 * P, SUP_Q), dtype=np.float32)
    for i in range(n):
        E = 512 * (i + 1)
        keys = E - 512 + np.arange(512)
        qrows = SUP_Q * g_map(n, i, h) + np.arange(SUP_Q)
        m[i] = (keys[:, None] <= qrows[None, :]).astype(np.float32)
    return m


def _gather_blocks(t_dcps, cols, nblk, width):
    """t_dcps: [C, P, S]; -> [nblk, P, C, width] for the given columns."""
    C = t_dcps.shape[0]
    g = t_dcps[:, :, cols].reshape(C, P, nblk, width)
    return np.ascontiguousarray(np.transpose(g, (2, 1, 0, 3)))


def prep_all(x, wq, wk, wv):
    import ml_dtypes
    bf = ml_dtypes.bfloat16
    B, SEQ, _ = x.shape
    pi = perm_indices()
    wqT_p = np.ascontiguousarray(wq[pi, :].T).astype(bf)
    wkT_p = np.ascontiguousarray(wk[pi, :].T).astype(bf)
    wvT = np.ascontiguousarray(wv.T).astype(bf)
    cos_t, sin_t = rope_tables(SEQ)
    cs = np.concatenate([cos_t.reshape(4, P, SEQ), sin_t.reshape(4, P, SEQ)],
                        axis=0).astype(bf)   # [8, P, SEQ]
    in_maps = []
    for c in range(2 * B):
        b, h = c // 2, c % 2
        xT = np.ascontiguousarray(x[b].T).astype(bf).reshape(DC, P, SEQ)
        qi = q_indices(SEQ, h)
        ki = k_indices(SEQ, h)
        mask = make_masks(SEQ, h).astype(bf)
        in_maps.append({
            "xk": _gather_blocks(xT, ki, NDB, 2 * W),
            "xq": _gather_blocks(xT, qi, NSUP, SUP_Q),
            "wkT": wkT_p, "wqT": wqT_p, "wvT": wvT,
            "csk": _gather_blocks(cs, ki, NDB, 2 * W),
            "csq": _gather_blocks(cs, qi, NSUP, SUP_Q),
            "mask": np.ascontiguousarray(
                mask.reshape(NSUP, 4, P, SUP_Q).transpose(0, 2, 1, 3)),
        })
    return in_maps


def assemble_output(results, B, SEQ):
    n = SEQ // (2 * SUP_Q)
    out = np.empty((B, SEQ, D), dtype=np.float32)
    for c in range(2 * B):
        b, h = c // 2, c % 2
        o = results[c]["out"]
        for i in range(n):
            g = g_map(n, i, h)
            out[b, SUP_Q * g:SUP_Q * (g + 1), :] = o[SUP_Q * i:SUP_Q * (i + 1), :]
    return out


# ---------------------------------------------------------------------------
# Entry point
# ---------------------------------------------------------------------------

_COMPILED = {}


def _get_compiled(SEQ, n_cores):
    key = (SEQ, n_cores)
    if key not in _COMPILED:
        nc = bacc.Bacc("TRN2", target_bir_lowering=False, debug=False,
                       num_devices=n_cores)
        build_kernel(nc, SEQ)
        nc.compile()
        _COMPILED[key] = nc
    return _COMPILED[key]


def kernel(x, wq, wk, wv):
    from concourse.bass_utils import run_bass_kernel_spmd
    x = np.asarray(x, dtype=np.float32)
    wq = np.asarray(wq, dtype=np.float32)
    wk = np.asarray(wk, dtype=np.float32)
    wv = np.asarray(wv, dtype=np.float32)
    B, SEQ, d = x.shape
    assert d == D
    n_cores = 2 * B
    nc = _get_compiled(SEQ, n_cores)
    in_maps = prep_all(x, wq, wk, wv)
    res = run_bass_kernel_spmd(nc, in_maps, list(range(n_cores)))
    return assemble_output(res.results, B, SEQ)


# revision 11
# speedup vs baseline: 1.0362x; 1.0362x over previous
"""Trainium2 Bass kernel: single-head causal attention with RoPE,
sharded across 8 NeuronCores (2 cores per batch element).

kernel(**inputs) takes the FULL inputs (x [4,4096,1024], wq/wk/wv
[1024,1024], all fp32) and returns the FULL output [4,4096,1024] fp32.

Sharding: core c handles batch b=c//2; parity h=c%2 selects both its
query superblocks (g_map pairing, identical extents 512*(i+1) on both
cores) and its key blocks (the 256-key blocks with global index 2j+h).
Each core projects K/V only for its own 8 key blocks (half the work),
processed as four 512-key double-blocks; each double-block's K^T/V is
exchanged pairwise with its own AllGather so the wire overlaps both the
remaining projections and the early attention superblocks.

Device layouts (partition dim first, rank-uniform so the program is
identical on every core):
  KT_sb [128, 8, 2, 8, 256]: K^T; dims = (feat%128, feat-chunk,
        rank-parity r, rank-local block B, key-in-block).  Global
        256-key block Gb = 2B + r.
  V_sb  [128, 2, 16, 1025]: V; dims = (key%128, rank-parity r,
        rank-local 128-key chunk 2B+half, feature).  Col 1024 = 1.0
        (softmax denominator rides the PV matmul).
  QT    [128, 8, 256] per-superblock Q^T (RoPE'd, perm)
Scores are computed transposed (S^T[k,q]) so P^T = exp(S^T) feeds the
PV matmul as the stationary operand.  Softmax runs without
max-subtraction (scores are O(6); exp is safe in fp32).

Engine budget: PE does all matmuls; DVE does RoPE directly from PSUM at
[128,512] granularity (and the V psum evictions) so the scalar engine
only runs input-stream DMAs and the phase-C exp activations; gpsimd
owns the collective triggers and the gather landing DMAs (interleaved
one chunk behind the triggers so a blocked landing never delays the
next trigger by more than one AG).
"""

import sys
for _p in ("/root/.axon_site", "/root/.axon_site/_ro/trn_rl_repo",
           "/root/.axon_site/_ro/pypackages"):
    if _p not in sys.path:
        sys.path.append(_p)

import numpy as np

import concourse.bass as bass
import concourse.bacc as bacc
import concourse.mybir as mybir
from concourse import tile

P = 128
D = 1024
DC = 8          # feature chunks of 128
W = 256         # key-block width
SUP_Q = 256     # q rows per superblock
NSUP = 8
NBLK = 8        # own 256-key blocks per core
NDB = 4         # own 512-key double-blocks per core (one AllGather each)
ROPE_BASE = 10000.0
SCALE = 1.0 / 32.0   # 1/sqrt(D)
DV1 = 1025      # V free width incl. ones column
PV_SPLIT = [(0, 342), (342, 684), (684, 1025)]
# KT column order inside a staged chunk: [se0, so0, se1, so1, ...] so each
# rope output pair is one contiguous [P,1024] store; landing looks up the
# feature-chunk position with this map.
KT_COL = [0, 2, 4, 6, 1, 3, 5, 7]

BF = mybir.dt.bfloat16
F32 = mybir.dt.float32
AF = mybir.ActivationFunctionType


def build_kernel(nc, SEQ):
    QROWS = SEQ // 2

    xk = nc.dram_tensor("xk", [NDB, P, DC, 2 * W], BF, kind="ExternalInput")
    xq = nc.dram_tensor("xq", [NSUP, P, DC, SUP_Q], BF, kind="ExternalInput")
    wkT = nc.dram_tensor("wkT", [D, D], BF, kind="ExternalInput")
    wqT = nc.dram_tensor("wqT", [D, D], BF, kind="ExternalInput")
    wvT = nc.dram_tensor("wvT", [D, D], BF, kind="ExternalInput")
    csk = nc.dram_tensor("csk", [NDB, P, 8, 2 * W], BF, kind="ExternalInput")
    csq = nc.dram_tensor("csq", [NSUP, P, 8, SUP_Q], BF, kind="ExternalInput")
    maskI = nc.dram_tensor("mask", [NSUP, P, 4, SUP_Q], BF, kind="ExternalInput")
    out = nc.dram_tensor("out", [QROWS, D], F32, kind="ExternalOutput")

    wk_r = wkT.rearrange("(c p) e -> p c e", p=P)
    wq_r = wqT.rearrange("(c p) e -> p c e", p=P)
    wv_r = wvT.rearrange("(c p) e -> p c e", p=P)

    # per-double-block AG bounce buffers:
    # cols [0:4096] = K^T (8 x 512 in KT_COL order), [4096:8192] = V
    couts = [nc.dram_tensor(f"cout{j}", [P, 8192], BF) for j in range(NDB)]
    cgs = [nc.dram_tensor(f"cg{j}", [2, P, 8192], BF) for j in range(NDB)]
    ngroups = max(1, nc.num_devices // 2)
    groups = [[2 * g, 2 * g + 1] for g in range(ngroups)]

    with tile.TileContext(nc) as tc:
        _emit(tc, nc, xk, xq, wk_r, wq_r, wv_r, csk, csq, maskI, out,
              couts, cgs, groups)
    return nc


def _rope_evict(nc, pool, pse, pso, cos_ap, sin_ap, out_e_ap, out_o_ap, width, tag):
    """out_e = e*cos - o*sin ; out_o = o*cos + e*sin, reading PSUM directly
    on the DVE (psum fp32 -> sbuf bf16)."""
    me = pool.tile([P, width], BF, tag=f"{tag}me")
    nc.vector.tensor_mul(me[:], pse[:], cos_ap)
    mo = pool.tile([P, width], BF, tag=f"{tag}mo")
    nc.vector.tensor_mul(mo[:], pso[:], sin_ap)
    nc.vector.tensor_sub(out_e_ap, me[:], mo[:])
    me2 = pool.tile([P, width], BF, tag=f"{tag}me")
    nc.vector.tensor_mul(me2[:], pse[:], sin_ap)
    mo2 = pool.tile([P, width], BF, tag=f"{tag}mo")
    nc.vector.tensor_mul(mo2[:], pso[:], cos_ap)
    nc.vector.tensor_add(out_o_ap, me2[:], mo2[:])


def _load_w_chunked(nc, pool, w_r, tag, split=False, chunks=None):
    # split=True: chunks 0-3 on the sync ring, 4-7 on the scalar ring so the
    # full weight arrives in half the time at startup
    tiles = []
    for dc in (chunks if chunks is not None else range(DC)):
        t = pool.tile([P, D], BF, tag=f"{tag}{dc}", name=f"{tag}_{dc}")
        eng = nc.scalar if (split and dc >= DC // 2) else nc.sync
        eng.dma_start(t[:], w_r[:, dc, :])
        tiles.append(t)
    return tiles


def _emit(tc, nc, xk, xq, wk_r, wq_r, wv_r, csk, csq, maskI, out,
          couts, cgs, groups):
    with (
        tc.tile_pool(name="kt", bufs=1) as ktp,
        tc.tile_pool(name="v", bufs=1) as vp,
        tc.tile_pool(name="wq", bufs=1) as wqp,
        tc.tile_pool(name="kvs", bufs=6) as kvsp,
    ):
        KT_sb = ktp.tile([P, DC, 2, NBLK, W], BF, tag="KT")
        V_sb = vp.tile([P, 2, 2 * NBLK, DV1], BF, tag="V")
        nc.vector.memset(V_sb[:, :, :, 1024:1025], 1.0)

        # ---------- Phase A: own-half K+V projections, chunked AllGather ----
        with (
            tc.tile_pool(name="wkv", bufs=1) as wkvp,
            tc.tile_pool(name="xs", bufs=2) as xsp,
            tc.tile_pool(name="cs", bufs=1) as csp,
            tc.tile_pool(name="ev", bufs=1) as evp,
            tc.tile_pool(name="pa", bufs=6, space="PSUM") as pap,
        ):
            xt0 = xsp.tile([P, DC, 2 * W], BF, tag="xk", name="xt_0")
            nc.scalar.dma_start(xt0[:], xk[0])
            cs0 = csp.tile([P, 8, 2 * W], BF, tag="cs", name="cs_0")
            nc.scalar.dma_start(cs0[:], csk[0])
            wk_t = _load_w_chunked(nc, wkvp, wk_r, "wk", split=True)
            wv_t = _load_w_chunked(nc, wkvp, wv_r, "wv", split=True)
            wq_lo = _load_w_chunked(nc, wqp, wq_r, "wq", chunks=range(4))

            def land(j):
                cgv = cgs[j].rearrange("g p x -> p g x")
                for r in range(2):
                    for jh in range(DC):
                        c0 = KT_COL[jh] * 512
                        nc.gpsimd.dma_start(
                            KT_sb[:, jh, r, 2 * j:2 * j + 2, :],
                            cgv[:, r, c0:c0 + 512])
                    nc.gpsimd.dma_start(
                        V_sb[:, r, 4 * j:4 * j + 4, 0:1024],
                        cgv[:, r, 4096:8192])

            for db in range(NDB):
                cout = couts[db]
                if db == 0:
                    xt, cst = xt0, cs0
                else:
                    xt = xsp.tile([P, DC, 2 * W], BF, tag="xk", name=f"xt_{db}")
                    nc.scalar.dma_start(xt[:], xk[db])
                    cst = csp.tile([P, 8, 2 * W], BF, tag="cs", name=f"cs_{db}")
                    nc.scalar.dma_start(cst[:], csk[db])

                # K projection: [P,512] psums, stationary shared across the
                # two 256-key halves
                for j in range(4):
                    pse = pap.tile([P, 2 * W], F32, tag="ps")
                    for dc in range(DC):
                        nc.tensor.matmul(pse[:],
                                         wk_t[dc][:, j * P:(j + 1) * P],
                                         xt[:, dc, :],
                                         start=(dc == 0), stop=(dc == DC - 1))
                    pso = pap.tile([P, 2 * W], F32, tag="ps")
                    for dc in range(DC):
                        nc.tensor.matmul(pso[:],
                                         wk_t[dc][:, (j + 4) * P:(j + 5) * P],
                                         xt[:, dc, :],
                                         start=(dc == 0), stop=(dc == DC - 1))
                    kso = kvsp.tile([P, 2, 2 * W], BF, tag="kso")
                    _rope_evict(nc, evp, pse, pso,
                                cst[:, j, :], cst[:, j + 4, :],
                                kso[:, 0, :], kso[:, 1, :], 2 * W, "k")
                    nc.sync.dma_start(
                        cout[:, j * 1024:(j + 1) * 1024], kso[:])

                # V projection
                for sc in range(4):
                    vso = kvsp.tile([P, 2, 512], BF, tag="kso")
                    for half in range(2):
                        psv = pap.tile([P, 512], F32, tag="ps")
                        for dc in range(DC):
                            nc.tensor.matmul(
                                psv[:],
                                xt[:, dc, sc * P:(sc + 1) * P],
                                wv_t[dc][:, half * 512:(half + 1) * 512],
                                start=(dc == 0), stop=(dc == DC - 1))
                        nc.vector.tensor_scalar_mul(vso[:, half, :], psv[:], 1.0)
                    base = 4096 + sc * 1024
                    nc.sync.dma_start(cout[:, base:base + 1024], vso[:])

                nc.gpsimd.collective_compute(
                    "AllGather", mybir.AluOpType.bypass,
                    replica_groups=groups, ins=[cout[:]], outs=[cgs[db][:]])
                # land the previous chunk's gather (one-chunk lag keeps the
                # next trigger from queueing behind a blocked landing)
                if db >= 1:
                    land(db - 1)
            land(NDB - 1)

        # ---------- Phase C: per-superblock Q projection + attention ----------
        with (
            tc.tile_pool(name="wqh", bufs=1) as wqhp,
            tc.tile_pool(name="xqp", bufs=2) as xqp,
            tc.tile_pool(name="cq", bufs=2) as cqp,
            tc.tile_pool(name="qt", bufs=2) as qtp,
            tc.tile_pool(name="evq", bufs=3) as evqp,
            tc.tile_pool(name="pt", bufs=6) as ptp,
            tc.tile_pool(name="mk", bufs=2) as mkp,
            tc.tile_pool(name="ot", bufs=3) as otp,
            tc.tile_pool(name="rd", bufs=2) as rdp,
            tc.tile_pool(name="pq", bufs=2, space="PSUM") as pqp,
            tc.tile_pool(name="po", bufs=1, space="PSUM") as pop,
        ):
            wq_hi = _load_w_chunked(nc, wqhp, wq_r, "wqh", split=True,
                                    chunks=range(4, DC))
            wq_t = list(wq_lo) + list(wq_hi)

            def load_slot(i):
                xqt = xqp.tile([P, DC, SUP_Q], BF, tag="xq")
                nc.scalar.dma_start(xqt[:], xq[i])
                cq = cqp.tile([P, 8, SUP_Q], BF, tag="cq")
                nc.scalar.dma_start(cq[:], csq[i])
                mk = mkp.tile([P, 4, SUP_Q], BF, tag="mk")
                nc.scalar.dma_start(mk[:], maskI[i])
                return xqt, cq, mk

            nxt = load_slot(0)
            for i in range(NSUP):
                q0 = i * SUP_Q
                xqt, cq, mk = nxt
                if i + 1 < NSUP:
                    nxt = load_slot(i + 1)
                QT = qtp.tile([P, DC, SUP_Q], BF, tag="QT")
                for j in range(4):
                    pse = pqp.tile([P, SUP_Q], F32, tag="pq")
                    for dc in range(DC):
                        nc.tensor.matmul(pse[:], wq_t[dc][:, j * P:(j + 1) * P],
                                         xqt[:, dc, :],
                                         start=(dc == 0), stop=(dc == DC - 1))
                    pso = pqp.tile([P, SUP_Q], F32, tag="pq")
                    for dc in range(DC):
                        nc.tensor.matmul(pso[:], wq_t[dc][:, (j + 4) * P:(j + 5) * P],
                                         xqt[:, dc, :],
                                         start=(dc == 0), stop=(dc == DC - 1))
                    _rope_evict(nc, evqp, pse, pso,
                                cq[:, j, :], cq[:, j + 4, :],
                                QT[:, j, :], QT[:, j + 4, :],
                                SUP_Q, "q")

                nkc = 4 * (i + 1)
                o_ps = [pop.tile([P, sl[1] - sl[0]], F32, tag=f"po{n}",
                                 name=f"o_ps{i}_{n}")
                        for n, sl in enumerate(PV_SPLIT + PV_SPLIT)]

                pending = None
                for kc in range(nkc):
                    gb, half = kc // 2, kc % 2
                    r, b = gb % 2, gb // 2
                    ps_s = pqp.tile([P, SUP_Q], F32, tag="pq")
                    for dc in range(DC):
                        nc.tensor.matmul(
                            ps_s[:],
                            KT_sb[:, dc, r, b, half * P:(half + 1) * P],
                            QT[:, dc, :],
                            start=(dc == 0), stop=(dc == DC - 1))
                    pt = ptp.tile([P, SUP_Q], BF, tag="pt")
                    nc.scalar.activation(pt[:], ps_s[:], AF.Exp, scale=SCALE)
                    if kc >= nkc - 4:
                        nc.vector.tensor_mul(pt[:], pt[:], mk[:, kc - (nkc - 4), :])
                    if pending is not None:
                        _emit_pv(nc, pending, V_sb, o_ps, nkc)
                    pending = (pt, kc)
                _emit_pv(nc, pending, V_sb, o_ps, nkc)

                rd = rdp.tile([P, 2], F32, tag="rd")
                nc.vector.reciprocal(rd[:, 0:1], o_ps[2][:, 340:341])
                nc.vector.reciprocal(rd[:, 1:2], o_ps[5][:, 340:341])
                for qs in range(2):
                    ot = otp.tile([P, D], F32, tag="ot")
                    for n, (a, b) in enumerate(PV_SPLIT):
                        bb = min(b, D)
                        nc.scalar.mul(ot[:, a:bb], o_ps[qs * 3 + n][:, 0:bb - a],
                                      rd[:, qs:qs + 1])
                    r0 = q0 + qs * P
                    nc.sync.dma_start(out[r0:r0 + P, :], ot[:])


def _emit_pv(nc, pending, V_sb, o_ps, nkc):
    pt, kc = pending
    first = (kc == 0)
    last = (kc == nkc - 1)
    gb, half = kc // 2, kc % 2
    r, b = gb % 2, gb // 2
    for qs in range(2):
        lhs = pt[:, qs * P:(qs + 1) * P]
        for n, (a, bb) in enumerate(PV_SPLIT):
            nc.tensor.matmul(o_ps[qs * 3 + n][:], lhs,
                             V_sb[:, r, 2 * b + half, a:bb],
                             start=first, stop=last)


# ---------------------------------------------------------------------------
# Host-side data preparation
# ---------------------------------------------------------------------------

def rope_tables(seq_len):
    pos = np.arange(seq_len, dtype=np.float64)
    inv = ROPE_BASE ** (-np.arange(0, D, 2, dtype=np.float64) / D)
    fr = inv[:, None] * pos[None, :]
    return np.cos(fr).astype(np.float32), np.sin(fr).astype(np.float32)


def perm_indices():
    return np.concatenate([np.arange(0, D, 2), np.arange(1, D, 2)])


def g_map(NSUP_, i, h):
    """Global superblock held in local slot i on parity-h cores.  The first
    half of slots takes parity h, the second half parity 1-h: both cores
    then need extent exactly 512*(i+1) per slot and the mask-rounding waste
    splits evenly instead of landing all on one parity."""
    return 2 * i + h if i < NSUP_ // 2 else 2 * i + (1 - h)


def q_indices(SEQ, h):
    n = SEQ // (2 * SUP_Q)
    return np.concatenate(
        [np.arange(SUP_Q * g_map(n, i, h), SUP_Q * g_map(n, i, h) + SUP_Q)
         for i in range(n)])


def k_indices(SEQ, h):
    """Own key columns: the 256-key blocks with global index 2j+h."""
    n = SEQ // (2 * W)
    return np.concatenate(
        [np.arange(W * (2 * j + h), W * (2 * j + h) + W) for j in range(n)])


def make_masks(SEQ, h):
    n = SEQ // (2 * SUP_Q)
    m = np.zeros((n, 4 * P, SUP_Q), dtype=np.float32)
    for i in range(n):
        E = 512 * (i + 1)
        keys = E - 512 + np.arange(512)
        qrows = SUP_Q * g_map(n, i, h) + np.arange(SUP_Q)
        m[i] = (keys[:, None] <= qrows[None, :]).astype(np.float32)
    return m


def _gather_blocks(t_dcps, cols, nblk, width):
    """t_dcps: [C, P, S]; -> [nblk, P, C, width] for the given columns."""
    C = t_dcps.shape[0]
    g = t_dcps[:, :, cols].reshape(C, P, nblk, width)
    return np.ascontiguousarray(np.transpose(g, (2, 1, 0, 3)))


def prep_all(x, wq, wk, wv):
    import ml_dtypes
    bf = ml_dtypes.bfloat16
    B, SEQ, _ = x.shape
    pi = perm_indices()
    wqT_p = np.ascontiguousarray(wq[pi, :].T).astype(bf)
    wkT_p = np.ascontiguousarray(wk[pi, :].T).astype(bf)
    wvT = np.ascontiguousarray(wv.T).astype(bf)
    cos_t, sin_t = rope_tables(SEQ)
    cs = np.concatenate([cos_t.reshape(4, P, SEQ), sin_t.reshape(4, P, SEQ)],
                        axis=0).astype(bf)   # [8, P, SEQ]
    in_maps = []
    for c in range(2 * B):
        b, h = c // 2, c % 2
        xT = np.ascontiguousarray(x[b].T).astype(bf).reshape(DC, P, SEQ)
        qi = q_indices(SEQ, h)
        ki = k_indices(SEQ, h)
        mask = make_masks(SEQ, h).astype(bf)
        in_maps.append({
            "xk": _gather_blocks(xT, ki, NDB, 2 * W),
            "xq": _gather_blocks(xT, qi, NSUP, SUP_Q),
            "wkT": wkT_p, "wqT": wqT_p, "wvT": wvT,
            "csk": _gather_blocks(cs, ki, NDB, 2 * W),
            "csq": _gather_blocks(cs, qi, NSUP, SUP_Q),
            "mask": np.ascontiguousarray(
                mask.reshape(NSUP, 4, P, SUP_Q).transpose(0, 2, 1, 3)),
        })
    return in_maps


def assemble_output(results, B, SEQ):
    n = SEQ // (2 * SUP_Q)
    out = np.empty((B, SEQ, D), dtype=np.float32)
    for c in range(2 * B):
        b, h = c // 2, c % 2
        o = results[c]["out"]
        for i in range(n):
            g = g_map(n, i, h)
            out[b, SUP_Q * g:SUP_Q * (g + 1), :] = o[SUP_Q * i:SUP_Q * (i + 1), :]
    return out


# ---------------------------------------------------------------------------
# Entry point
# ---------------------------------------------------------------------------

_COMPILED = {}


def _get_compiled(SEQ, n_cores):
    key = (SEQ, n_cores)
    if key not in _COMPILED:
        nc = bacc.Bacc("TRN2", target_bir_lowering=False, debug=False,
                       num_devices=n_cores)
        build_kernel(nc, SEQ)
        nc.compile()
        _COMPILED[key] = nc
    return _COMPILED[key]


def kernel(x, wq, wk, wv):
    from concourse.bass_utils import run_bass_kernel_spmd
    x = np.asarray(x, dtype=np.float32)
    wq = np.asarray(wq, dtype=np.float32)
    wk = np.asarray(wk, dtype=np.float32)
    wv = np.asarray(wv, dtype=np.float32)
    B, SEQ, d = x.shape
    assert d == D
    n_cores = 2 * B
    nc = _get_compiled(SEQ, n_cores)
    in_maps = prep_all(x, wq, wk, wv)
    res = run_bass_kernel_spmd(nc, in_maps, list(range(n_cores)))
    return assemble_output(res.results, B, SEQ)
